# revision 30
# baseline (speedup 1.0000x reference)
"""Trainium2 Bass kernel: 3-layer actor MLP over [B=256, K=1000] actions.

Math (per reference):
    h1 = relu(af @ W1_a + state @ W1_s + b1)   # [B,K,256]
    h2 = relu(h1 @ W2 + b2)                    # [B,K,128]
    out = h2 @ W3 + b3                         # [B,K]

Sharding: data-parallel over B across 8 NeuronCores (32 rows each);
weights replicated.  Compute in bf16 (f32 PSUM accumulate).

Measured bottleneck: PSUM evacuation, not matmul.  On TRN2 only
ScalarE (1 elem/cyc/lane @1.2GHz, ~310cyc/op overhead) and VectorE
(1 elem/cyc/lane @0.96GHz, ~210cyc/op) can read PSUM (GpSimd and DMA
have no PSUM route), so the per-step relu+bias evacuation of h1
(2 x 1000 lane-elems) + h2 (2 x 500) sets the ~1.90us step cadence;
TensorE needs only ~1.8us.  Design:
  * L1's two k-chunk matmuls write one [128,1024] PSUM bank-pair per
    h-half, chunk0 at bank offset 12 so both chunks' real data is one
    contiguous [12:1012] span; ONE merged contiguous [128,1000]
    relu+bias epilogue per h-half (zero pad cols) amortizes the fixed
    op overhead.  2-window pad-skipping APs and col-tiled L3 matmuls
    were both tried and are SLOWER / broken (interleaved accumulation
    groups on one bank silently drop accumulating writes on HW).
    During the first 3 (cold-clock) steps the L1 eps run unmerged as
    2x500 on opposite engines: the merged ep is a ~3us PSUM-recycle
    ring through a cold ScalarE and stalls the ramp.
  * Engine split: ScE gets h0-merged + L2c0 (~1.78us), DVE gets
    h1-merged + L2c1 (~1.93us).  Asymmetric k-chunks that would
    perfectly balance need >512 f32 per PSUM bank -> impossible.
  * L2 uses single-bank tiles from a bufs=3 pool: sub-slices of one
    merged tile serialize (W-after-R dependency tracking is
    tile-granular), and bufs=3 gives 1.5-step ring slack so the next
    step's matmuls don't wait on this step's epilogues.
  * PSUM budget (8 banks): L1 2x[128,1024] (4) + L2 3x[128,512] (3)
    + L3 1.
  * L3: score row (b, ch) lands on PSUM partition p = 2bh+ch via a
    full-array matmul whose stationary is zeros except col p = w3
    (sliding strip window).  TWO sequential 16-step accumulation
    groups reuse the single L3 bank (pool W-after-R covers the
    handoff), so the first half's copy + 64KB output DMA overlap
    steady state at step ~17; only the second half's copy + DMA sit
    on the tail (b3 added on host).
  * Startup DMA: consts layout puts L1-critical w1a first and the
    consts transfer is split 832/1024 cols, so L1(b0) gates on only
    ~725KB (all 8 cores hit HBM at once at startup).
  * Ramp: the PE clock governor only reaches 2.4GHz after ~4us of
    sustained FULL-ARRAY matmul traffic (64-row dummies leave 3/4 of
    the PE idle and never trip it; fewer than ~11 dummies makes it
    oscillate), so the warm-up runs 11 full-array
    512-col dummies while the first input DMAs land; input groups are
    GRP=4 rows (0.5MB), and group 0 is split into four per-row 128KB
    DMAs so step 0 never waits on a late bulk transfer; all DMA
    issues precede every other engine op.  Starting real steps before
    the governor warms (fewer dummies) measures WORSE: cold steps run
    2x slow and the governor ramps on its own ~7us timeline.

Device tensors per core (host pre-packs, all contiguous):
  a2     [8,128,2048] bf16  action_feats.T in contiguous 4-row groups
                            (per row: p 0:64 = k 0:500, p 64:128 =
                            k 500:1000; cols 500:512 zero pad) so each
                            input DMA is a single contiguous transfer
  consts [128,1856]   bf16  w1a(dup)|w2|w3strip|state.T|w1s(4x128 chunks)
                            (L1-critical w1a first; DMA split 832|1024)
  biases [128,3]      f32   b1 (2 cols) | b2  (b3 added host-side)
  out    [32,1000]    f32   final scores (host adds b3 only)
"""

import os
import numpy as np

B, K = 256, 1000
SD, AD, H, G = 512, 64, 256, 128
NCORES = 8
BL = B // NCORES          # 32 batch rows per core
KC = 500                  # real k-chunk length (2 chunks per row)
KP = 512                  # padded k-chunk length (fills one PSUM bank)
EPW = 2 * KP - (KP - KC)  # 1012: merged-epilogue width (skip last pad)
GRP = 4                   # batch rows per input-DMA group
NCONST = 1856             # bf16 cols: w1s|s2|w1a|w2|w3strip

_CACHE = {}
LAST_EXEC_NS = None


def _build_nc():
    from contextlib import ExitStack

    import concourse.bass as bass
    import concourse.bacc as bacc
    import concourse.mybir as mybir
    import concourse.tile as tile

    f32 = mybir.dt.float32
    bf16 = mybir.dt.bfloat16
    AF = mybir.ActivationFunctionType
    ALU = mybir.AluOpType

    nc = bacc.Bacc("TRN2", target_bir_lowering=False, debug=False,
                   num_devices=NCORES)

    a2 = nc.dram_tensor("a2", [BL // GRP, 128, GRP * KP], bf16,
                        kind="ExternalInput").ap()
    constsd = nc.dram_tensor("consts", [128, NCONST], bf16,
                             kind="ExternalInput").ap()
    biasd = nc.dram_tensor("biases", [128, 3], f32, kind="ExternalInput").ap()
    out = nc.dram_tensor("out", [BL, 2 * KC], f32,
                         kind="ExternalOutput").ap()

    with tile.TileContext(nc) as tc, ExitStack() as ctx:
        wp = ctx.enter_context(tc.tile_pool(name="wp", bufs=1))
        xp = ctx.enter_context(tc.tile_pool(name="xp", bufs=3))
        xrp = ctx.enter_context(tc.tile_pool(name="xrp", bufs=GRP))
        h1p = ctx.enter_context(tc.tile_pool(name="h1p", bufs=6))
        h2p = ctx.enter_context(tc.tile_pool(name="h2p", bufs=3))
        osp = ctx.enter_context(tc.tile_pool(name="osp", bufs=1))
        # pp: L1 merged h-half tiles (2 banks each, 4 banks); l2p: L2
        # single-bank chunk tiles (3 banks, 1.5-step ring slack); l3p: 1.
        pp = ctx.enter_context(tc.tile_pool(name="pp", bufs=2, space="PSUM"))
        l2p = ctx.enter_context(tc.tile_pool(name="l2p", bufs=3, space="PSUM"))
        l3p = ctx.enter_context(tc.tile_pool(name="l3p", bufs=1, space="PSUM"))

        # ---- DMAs first: consts + first input groups are the critical
        # path to step 0; issue before any other engine work ----
        # consts layout puts the L1-critical w1a first; the transfer is
        # split so L1(b0) only gates on 213KB of consts (all 8 cores DMA
        # at once at startup -- less on the critical path matters).
        cs = wp.tile([128, NCONST], bf16)
        nc.sync.dma_start(cs[:, 0:832], constsd[:, 0:832])
        w1a_sb = cs[:, 0:256]
        w2_sb = cs[:, 256:512].rearrange("p (c g) -> p c g", c=2)
        w3s_sb = cs[:, 512:704]
        s2_sb = cs[:, 704:832]
        w1s_sb = cs[:, 832:1856].rearrange("p (c h) -> p c h", c=4)

        xts = {}
        xrow = {}

        def stage_xt(g):
            if g < BL // GRP and g not in xts:
                xt = xp.tile([128, GRP, KP], bf16, tag="xt")
                xts[g] = xt
                nc.sync.dma_start(
                    xts[g][:].rearrange("p j y -> p (j y)"), a2[g, :, :])

        # group 0 as per-row tiles: the first step gates on one 128KB DMA
        # (8 cores contend for HBM at startup; row 0 lands ~4us earlier
        # than the whole group would)
        xts[0] = True
        for j in range(GRP):
            xr = xrp.tile([128, KP], bf16, tag="xtr", name=f"xr{j}")
            xrow[j] = xr
            nc.sync.dma_start(xr[:], a2[0, :, KP * j:KP * (j + 1)])
        bb = wp.tile([128, 3], f32)
        nc.sync.dma_start(bb[:], biasd[:])
        b1_sb = bb[:, 0:2]
        b2_sb = bb[:, 2:3]
        nc.sync.dma_start(cs[:, 832:1856], constsd[:, 832:1856])
        stage_xt(1)

        # ---- ACT table preload: fire Relu once so the 1.3us table load
        # overlaps the input DMAs instead of blocking the first epilogue ----
        da = wp.tile([128, 2], f32)
        nc.gpsimd.memset(da[:], 0.0)
        dummy = wp.tile([128, 640], bf16)
        nc.gpsimd.memset(dummy[:], 0.0)
        nc.scalar.activation(da[:, 1:2], da[:, 0:1], AF.Relu)

        # ---- epilogue engine balancer (ACT vs DVE), HW cost models ----
        eng_ns = [0.0, 0.0]

        def ep(out_ap, in_ap, bias_ap, relu, eng=None):
            fd = in_ap.free_size()
            cost_act = (fd + 310) / 1.2
            cost_dve = (fd + 120) / 0.96 + 90
            if eng is None:
                eng = 0 if eng_ns[0] + cost_act <= eng_ns[1] + cost_dve else 1
            with tc.high_priority():
                if eng == 0:
                    eng_ns[0] += cost_act
                    return nc.scalar.activation(
                        out_ap, in_ap, AF.Relu if relu else AF.Identity,
                        bias=bias_ap)
                eng_ns[1] += cost_dve
                if relu:
                    return nc.vector.tensor_scalar(out_ap, in_ap, bias_ap,
                                                   0.0, ALU.add, ALU.max)
                return nc.vector.tensor_scalar(out_ap, in_ap, bias_ap, None,
                                               ALU.add)

        # ---- PE warm-up while the first DMAs land: FULL-ARRAY matmuls
        # (a 64x64 stationary leaves 3/4 of the PE idle and never trips
        # the HAM governor into the 2.4GHz K=8/8 state) ----
        wps = pp.tile([128, 1024], f32, tag="pp")
        for _ in range(11):
            nc.tensor.matmul(wps[:, 0:512], lhsT=dummy[:, 512:640],
                             rhs=dummy[:, 0:512], start=True, stop=True)

        # ---- h_state = (state @ W1_s).T + b1 : [128, 64], col 32h+b ----
        # (transient pp PSUM + DVE-forced epilogue so neither blocks L1(b0))
        hs_sb = wp.tile([128, 2 * BL], f32)
        hsps = l2p.tile([128, 512], f32, tag="l2")
        for h in range(2):
            for c in range(4):
                nc.tensor.matmul(
                    hsps[:, 32 * h:32 * (h + 1)],
                    lhsT=w1s_sb[:, c, 128 * h:128 * (h + 1)],
                    rhs=s2_sb[:, 32 * c:32 * (c + 1)],
                    start=(c == 0), stop=(c == 3))
            with tc.high_priority():
                nc.scalar.activation(
                    hs_sb[:, 32 * h:32 * (h + 1)],
                    hsps[:, 32 * h:32 * (h + 1)],
                    AF.Identity, bias=b1_sb[:, h:h + 1])
                eng_ns[0] += 290

        # ---- per-step stages (software-pipelined: L1(s), L2(s-1),
        # L3(s-2)) ----

        def stage_l1h(b, h):
            # one [128,1024] PSUM bank-pair; the two k-chunk matmuls run
            # concurrently on row-groups 0:64 / 64:128, then ONE merged
            # relu+bias epilogue evacuates both banks in a single op.
            g, j = divmod(b, GRP)
            h1 = h1p.tile([128, 1024], bf16, tag="h1")
            l1t = pp.tile([128, 1024], f32, tag="pp")
            # c0 lands at bank offset 12 so the real data of both chunks
            # is one contiguous [12:1012] span: the merged epilogue covers
            # 1000 elems with no pad cols, and only real cols are streamed.
            for c in range(2):
                nc.tensor.matmul(
                    l1t[:, 12 + 500 * c:12 + 500 * (c + 1)],
                    lhsT=w1a_sb[64 * c:64 * (c + 1),
                                128 * h:128 * (h + 1)],
                    rhs=(xrow[b][64 * c:64 * (c + 1), 0:KC] if g == 0 else
                         xts[g][64 * c:64 * (c + 1), j, 0:KC]),
                    start=True, stop=True)
            bias = hs_sb[:, 32 * h + b:32 * h + b + 1]
            if b < 3:
                # cold-clock phase: the merged ep is a ~3us ring through a
                # cold ScalarE; two 500-col eps on OPPOSITE engines run in
                # parallel and halve the ring until the governor warms up.
                for c in range(2):
                    ep(h1[:, 12 + 500 * c:12 + 500 * (c + 1)],
                       l1t[:, 12 + 500 * c:12 + 500 * (c + 1)],
                       bias, relu=True, eng=(h + c) % 2)
            else:
                ep(h1[:, 12:EPW], l1t[:, 12:EPW], bias, relu=True, eng=h)
            return h1

        def stage_l2(b, h1t):
            # merged [128,1024] PSUM tile; hh-outer so each W2 half is one
            # LDWEIGHTS; ONE merged relu+bias epilogue.
            h2 = h2p.tile([128, 1024], bf16, tag="h2")
            for c in range(2):
                l2t = l2p.tile([128, 512], f32, tag="l2")
                for hh in range(2):
                    nc.tensor.matmul(
                        l2t[:, 0:KC],
                        lhsT=w2_sb[:, hh, :],
                        rhs=h1t[hh][:, 12 + 500 * c:12 + 500 * (c + 1)],
                        start=(hh == 0), stop=(hh == 1))
                # per-chunk single-bank tile + epilogue right after its
                # 2-matmul group closes: independent tiles keep the ring
                # short (tile-granularity W-after-R tracking serializes
                # sub-slices of one tile).
                ep(h2[:, KP * c:KP * c + KC], l2t[:, 0:KC],
                   b2_sb[:, 0:1], relu=True, eng=c)
            return h2

        # L3: score row (b, ch) lands on PSUM partition p = 2b+ch via a
        # full-array matmul whose stationary is zeros except col p = w3
        # (sliding window of w3s_sb); all 64 matmuls accumulate into one
        # bank (+0.0 on every other partition), one copy + one DMA out.
        # Two sequential 16-step accumulation groups reuse the single L3
        # bank (pool W-after-R covers the handoff), so the first half's
        # copy + 64KB DMA overlap steady state at step ~17 instead of
        # sitting on the tail.
        l3state = {}

        def stage_l3(b, h2):
            half, bh = divmod(b, BL // 2)
            if bh == 0:
                l3state[0] = l3p.tile([128, 512], f32, tag="l3",
                                      name=f"l3bank{half}")
            l3b = l3state[0]
            for ch in range(2):
                p = 2 * bh + ch
                nc.tensor.matmul(
                    l3b[:, 0:KC],
                    lhsT=w3s_sb[:, 63 - p:191 - p],
                    rhs=h2[:, 0:KC] if ch == 0 else h2[:, KP:KP + KC],
                    start=(p == 0), stop=(p == BL - 1))
            if bh == BL // 2 - 1:
                if half == 0:
                    l3state["osb"] = osp.tile([2 * BL, 500], f32, tag="osb",
                                              name="osb")
                osb = l3state["osb"]
                rows = slice(BL * half, BL * (half + 1))
                with tc.high_priority():
                    if half == 0:
                        # mid-run: ScE identity copy (its slack absorbs it)
                        nc.scalar.activation(osb[rows, :], l3b[0:BL, 0:KC],
                                             AF.Identity)
                    else:
                        nc.vector.tensor_scalar(osb[rows, :],
                                                l3b[0:BL, 0:KC], 0.0,
                                                None, ALU.add)
                nc.sync.dma_start(
                    out[16 * half:16 * (half + 1)]
                    .rearrange("b (c y) -> (b c) y", c=2), osb[rows, :])

        pend = {}
        pend2 = {}
        for s in range(BL + 2):
            if s < BL:
                if s % GRP == 0:
                    stage_xt(s // GRP + 1)
                h1a = stage_l1h(s, 0)
                h1b = stage_l1h(s, 1)
            if s - 1 in pend:
                pend2[s - 1] = stage_l2(s - 1, pend.pop(s - 1))
            if s - 2 in pend2:
                stage_l3(s - 2, pend2.pop(s - 2))
            if s < BL:
                pend[s] = [h1a, h1b]

    nc.compile()
    return nc


def _prep_inputs(state_embed, action_feats, W1, b1, W2, b2, W3, b3):
    import ml_dtypes
    bf = ml_dtypes.bfloat16
    f4 = lambda x: np.ascontiguousarray(np.asarray(x, dtype=np.float32))
    state_embed, action_feats = f4(state_embed), f4(action_feats)
    W1, b1, W2, b2, W3, b3 = map(f4, (W1, b1, W2, b2, W3, b3))

    W1s, W1a = W1[:SD], W1[SD:]
    w1s_h = np.concatenate([W1s[c * 128:(c + 1) * 128] for c in range(4)],
                           axis=1).astype(bf)                # [128, 1024]
    w1a_h = np.concatenate([W1a, W1a], axis=0).astype(bf)    # [128, 256]
    w2_h = np.concatenate([W2[:128], W2[128:]], axis=1).astype(bf)
    w3s_h = np.zeros((G, 192), dtype=np.float32)
    w3s_h[:, 63] = W3[:, 0]
    w3s_h = w3s_h.astype(bf)
    biases = np.ascontiguousarray(np.concatenate(
        [b1.reshape(2, 128).T, b2.reshape(G, 1)], axis=1))   # [128, 3] f32

    in_maps = []
    for ci in range(NCORES):
        sl = slice(ci * BL, (ci + 1) * BL)
        aft = action_feats[sl].transpose(0, 2, 1)            # [BL, 64, 1000]
        a2_h = np.zeros((BL, 128, KP), dtype=bf)
        a2_h[:, 0:64, 0:KC] = aft[:, :, :KC].astype(bf)
        a2_h[:, 64:128, 0:KC] = aft[:, :, KC:].astype(bf)
        a2_h = np.ascontiguousarray(
            a2_h.reshape(BL // GRP, GRP, 128, KP).transpose(0, 2, 1, 3)
            .reshape(BL // GRP, 128, GRP * KP))
        st = state_embed[sl].T.astype(bf)                    # [512, BL]
        s2_h = np.concatenate([st[c * 128:(c + 1) * 128] for c in range(4)],
                              axis=1)                        # [128, 128]
        consts = np.ascontiguousarray(np.concatenate(
            [w1a_h, w2_h, w3s_h, s2_h, w1s_h], axis=1))
        assert consts.shape == (128, NCONST), consts.shape
        in_maps.append({"a2": a2_h, "consts": consts, "biases": biases})
    return in_maps, float(b3.reshape(-1)[0])


def kernel(state_embed, action_feats, W1, b1, W2, b2, W3, b3):
    global LAST_EXEC_NS
    from concourse.bass_utils import run_bass_kernel_spmd

    if "nc" not in _CACHE:
        _CACHE["nc"] = _build_nc()
    nc = _CACHE["nc"]

    in_maps, b3v = _prep_inputs(state_embed, action_feats, W1, b1, W2, b2,
                                W3, b3)
    trace = bool(int(os.environ.get("ACTOR_KERNEL_TRACE", "0")))
    res = run_bass_kernel_spmd(nc, in_maps, core_ids=list(range(NCORES)),
                               trace=trace)
    LAST_EXEC_NS = res.exec_time_ns
    outs = [np.asarray(res.results[i]["out"]) for i in range(NCORES)]
    return np.ascontiguousarray(
        (np.concatenate(outs, axis=0) + b3v).astype(np.float32))


# revision 31
# speedup vs baseline: 1.0002x; 1.0002x over previous
"""Trainium2 Bass kernel: 3-layer actor MLP over [B=256, K=1000] actions.

Math (per reference):
    h1 = relu(af @ W1_a + state @ W1_s + b1)   # [B,K,256]
    h2 = relu(h1 @ W2 + b2)                    # [B,K,128]
    out = h2 @ W3 + b3                         # [B,K]

Sharding: data-parallel over B across 8 NeuronCores (32 rows each);
weights replicated.  Compute in bf16 (f32 PSUM accumulate).

Measured bottleneck: PSUM evacuation, not matmul.  On TRN2 only
ScalarE (1 elem/cyc/lane @1.2GHz, ~310cyc/op overhead) and VectorE
(1 elem/cyc/lane @0.96GHz, ~210cyc/op) can read PSUM (GpSimd and DMA
have no PSUM route), so the per-step relu+bias evacuation of h1
(2 x 1000 lane-elems) + h2 (2 x 500) sets the ~1.90us step cadence;
TensorE needs only ~1.8us.  Design:
  * L1's two k-chunk matmuls write one [128,1024] PSUM bank-pair per
    h-half, chunk0 at bank offset 12 so both chunks' real data is one
    contiguous [12:1012] span; ONE merged contiguous [128,1000]
    relu+bias epilogue per h-half (zero pad cols) amortizes the fixed
    op overhead.  2-window pad-skipping APs and col-tiled L3 matmuls
    were both tried and are SLOWER / broken (interleaved accumulation
    groups on one bank silently drop accumulating writes on HW).
    During the first 3 (cold-clock) steps the L1 eps run unmerged as
    2x500 on opposite engines: the merged ep is a ~3us PSUM-recycle
    ring through a cold ScalarE and stalls the ramp.
  * Engine split: ScE gets h0-merged + L2c0 (~1.78us), DVE gets
    h1-merged + L2c1 (~1.93us).  Asymmetric k-chunks that would
    perfectly balance need >512 f32 per PSUM bank -> impossible.
  * L2 uses single-bank tiles from a bufs=3 pool: sub-slices of one
    merged tile serialize (W-after-R dependency tracking is
    tile-granular), and bufs=3 gives 1.5-step ring slack so the next
    step's matmuls don't wait on this step's epilogues.
  * PSUM budget (8 banks): L1 2x[128,1024] (4) + L2 3x[128,512] (3)
    + L3 1.
  * L3: score row (b, ch) lands on PSUM partition p = 2bh+ch via a
    full-array matmul whose stationary is zeros except col p = w3
    (sliding strip window).  TWO sequential 16-step accumulation
    groups reuse the single L3 bank (pool W-after-R covers the
    handoff), so the first half's copy + 64KB output DMA overlap
    steady state at step ~17; only the second half's copy + DMA sit
    on the tail (b3 added on host).
  * Startup DMA: consts layout puts L1-critical w1a first and the
    consts transfer is split 832/1024 cols, so L1(b0) gates on only
    ~725KB (all 8 cores hit HBM at once at startup).
  * Ramp: the PE clock governor only reaches 2.4GHz after ~4us of
    sustained FULL-ARRAY matmul traffic (64-row dummies leave 3/4 of
    the PE idle and never trip it; fewer than ~11 dummies makes it
    oscillate), so the warm-up runs 11 full-array
    512-col dummies while the first input DMAs land; input groups are
    GRP=4 rows (0.5MB), and group 0 is split into four per-row 128KB
    DMAs so step 0 never waits on a late bulk transfer; all DMA
    issues precede every other engine op.  Starting real steps before
    the governor warms (fewer dummies) measures WORSE: cold steps run
    2x slow and the governor ramps on its own ~7us timeline.

Device tensors per core (host pre-packs, all contiguous):
  a2     [8,128,2048] bf16  action_feats.T in contiguous 4-row groups
                            (per row: p 0:64 = k 0:500, p 64:128 =
                            k 500:1000; cols 500:512 zero pad) so each
                            input DMA is a single contiguous transfer
  consts [128,1856]   bf16  w1a(dup)|w2|w3strip|state.T|w1s(4x128 chunks)
                            (L1-critical w1a first; DMA split 832|1024)
  biases [128,3]      f32   b1 (2 cols) | b2  (b3 added host-side)
  out    [32,1000]    f32   final scores (host adds b3 only)
"""

import os
import numpy as np

B, K = 256, 1000
SD, AD, H, G = 512, 64, 256, 128
NCORES = 8
BL = B // NCORES          # 32 batch rows per core
KC = 500                  # real k-chunk length (2 chunks per row)
KP = 512                  # padded k-chunk length (fills one PSUM bank)
EPW = 2 * KP - (KP - KC)  # 1012: merged-epilogue width (skip last pad)
GRP = 4                   # batch rows per input-DMA group
NCONST = 1856             # bf16 cols: w1s|s2|w1a|w2|w3strip

_CACHE = {}
LAST_EXEC_NS = None


def _build_nc():
    from contextlib import ExitStack

    import concourse.bass as bass
    import concourse.bacc as bacc
    import concourse.mybir as mybir
    import concourse.tile as tile

    f32 = mybir.dt.float32
    bf16 = mybir.dt.bfloat16
    AF = mybir.ActivationFunctionType
    ALU = mybir.AluOpType

    nc = bacc.Bacc("TRN2", target_bir_lowering=False, debug=False,
                   num_devices=NCORES)

    a2 = nc.dram_tensor("a2", [BL // GRP, 128, GRP * KP], bf16,
                        kind="ExternalInput").ap()
    constsd = nc.dram_tensor("consts", [128, NCONST], bf16,
                             kind="ExternalInput").ap()
    biasd = nc.dram_tensor("biases", [128, 3], f32, kind="ExternalInput").ap()
    out = nc.dram_tensor("out", [BL, 2 * KC], f32,
                         kind="ExternalOutput").ap()

    with tile.TileContext(nc) as tc, ExitStack() as ctx:
        wp = ctx.enter_context(tc.tile_pool(name="wp", bufs=1))
        xp = ctx.enter_context(tc.tile_pool(name="xp", bufs=3))
        xrp = ctx.enter_context(tc.tile_pool(name="xrp", bufs=GRP))
        h1p = ctx.enter_context(tc.tile_pool(name="h1p", bufs=6))
        h2p = ctx.enter_context(tc.tile_pool(name="h2p", bufs=3))
        osp = ctx.enter_context(tc.tile_pool(name="osp", bufs=1))
        # pp: L1 merged h-half tiles (2 banks each, 4 banks); l2p: L2
        # single-bank chunk tiles (3 banks, 1.5-step ring slack); l3p: 1.
        pp = ctx.enter_context(tc.tile_pool(name="pp", bufs=2, space="PSUM"))
        l2p = ctx.enter_context(tc.tile_pool(name="l2p", bufs=3, space="PSUM"))
        l3p = ctx.enter_context(tc.tile_pool(name="l3p", bufs=1, space="PSUM"))

        # ---- DMAs first: consts + first input groups are the critical
        # path to step 0; issue before any other engine work ----
        # consts layout puts the L1-critical w1a first; the transfer is
        # split so L1(b0) only gates on 213KB of consts (all 8 cores DMA
        # at once at startup -- less on the critical path matters).
        cs = wp.tile([128, NCONST], bf16)
        nc.sync.dma_start(cs[:, 0:832], constsd[:, 0:832])
        w1a_sb = cs[:, 0:256]
        w2_sb = cs[:, 256:512].rearrange("p (c g) -> p c g", c=2)
        w3s_sb = cs[:, 512:704]
        s2_sb = cs[:, 704:832]
        w1s_sb = cs[:, 832:1856].rearrange("p (c h) -> p c h", c=4)

        xts = {}
        xrow = {}

        def stage_xt(g):
            if g < BL // GRP and g not in xts:
                xt = xp.tile([128, GRP, KP], bf16, tag="xt")
                xts[g] = xt
                nc.sync.dma_start(
                    xts[g][:].rearrange("p j y -> p (j y)"), a2[g, :, :])

        # group 0 as per-row tiles: the first step gates on one 128KB DMA
        # (8 cores contend for HBM at startup; row 0 lands ~4us earlier
        # than the whole group would)
        xts[0] = True
        for j in range(GRP):
            xr = xrp.tile([128, KP], bf16, tag="xtr", name=f"xr{j}")
            xrow[j] = xr
            nc.sync.dma_start(xr[:], a2[0, :, KP * j:KP * (j + 1)])
        bb = wp.tile([128, 3], f32)
        nc.sync.dma_start(bb[:], biasd[:])
        b1_sb = bb[:, 0:2]
        b2_sb = bb[:, 2:3]
        nc.sync.dma_start(cs[:, 832:1856], constsd[:, 832:1856])
        stage_xt(1)

        # ---- ACT table preload: fire Relu once so the 1.3us table load
        # overlaps the input DMAs instead of blocking the first epilogue ----
        da = wp.tile([128, 2], f32)
        nc.gpsimd.memset(da[:], 0.0)
        dummy = wp.tile([128, 640], bf16)
        nc.gpsimd.memset(dummy[:], 0.0)
        nc.scalar.activation(da[:, 1:2], da[:, 0:1], AF.Relu)

        # ---- epilogue engine balancer (ACT vs DVE), HW cost models ----
        eng_ns = [0.0, 0.0]

        def ep(out_ap, in_ap, bias_ap, relu, eng=None):
            fd = in_ap.free_size()
            cost_act = (fd + 310) / 1.2
            cost_dve = (fd + 120) / 0.96 + 90
            if eng is None:
                eng = 0 if eng_ns[0] + cost_act <= eng_ns[1] + cost_dve else 1
            with tc.high_priority():
                if eng == 0:
                    eng_ns[0] += cost_act
                    return nc.scalar.activation(
                        out_ap, in_ap, AF.Relu if relu else AF.Identity,
                        bias=bias_ap)
                eng_ns[1] += cost_dve
                if relu:
                    return nc.vector.tensor_scalar(out_ap, in_ap, bias_ap,
                                                   0.0, ALU.add, ALU.max)
                return nc.vector.tensor_scalar(out_ap, in_ap, bias_ap, None,
                                               ALU.add)

        # ---- PE warm-up while the first DMAs land: FULL-ARRAY matmuls
        # (a 64x64 stationary leaves 3/4 of the PE idle and never trips
        # the HAM governor into the 2.4GHz K=8/8 state) ----
        # The hstate block is interleaved MID-warmup: L1(b0)'s start is
        # unchanged (same PE work ahead of it in the FIFO) but the hs_sb
        # bias ACTs finish ~1.5us earlier, unblocking step 0's epilogues.
        wps = pp.tile([128, 1024], f32, tag="pp")
        for _ in range(6):
            nc.tensor.matmul(wps[:, 0:512], lhsT=dummy[:, 512:640],
                             rhs=dummy[:, 0:512], start=True, stop=True)

        # ---- h_state = (state @ W1_s).T + b1 : [128, 64], col 32h+b ----
        hs_sb = wp.tile([128, 2 * BL], f32)
        hsps = l2p.tile([128, 512], f32, tag="l2")
        for h in range(2):
            for c in range(4):
                nc.tensor.matmul(
                    hsps[:, 32 * h:32 * (h + 1)],
                    lhsT=w1s_sb[:, c, 128 * h:128 * (h + 1)],
                    rhs=s2_sb[:, 32 * c:32 * (c + 1)],
                    start=(c == 0), stop=(c == 3))
            with tc.high_priority():
                nc.scalar.activation(
                    hs_sb[:, 32 * h:32 * (h + 1)],
                    hsps[:, 32 * h:32 * (h + 1)],
                    AF.Identity, bias=b1_sb[:, h:h + 1])
                eng_ns[0] += 290

        for _ in range(5):
            nc.tensor.matmul(wps[:, 0:512], lhsT=dummy[:, 512:640],
                             rhs=dummy[:, 0:512], start=True, stop=True)

        # ---- per-step stages (software-pipelined: L1(s), L2(s-1),
        # L3(s-2)) ----

        def stage_l1h(b, h):
            # one [128,1024] PSUM bank-pair; the two k-chunk matmuls run
            # concurrently on row-groups 0:64 / 64:128, then ONE merged
            # relu+bias epilogue evacuates both banks in a single op.
            g, j = divmod(b, GRP)
            h1 = h1p.tile([128, 1024], bf16, tag="h1")
            l1t = pp.tile([128, 1024], f32, tag="pp")
            # c0 lands at bank offset 12 so the real data of both chunks
            # is one contiguous [12:1012] span: the merged epilogue covers
            # 1000 elems with no pad cols, and only real cols are streamed.
            for c in range(2):
                nc.tensor.matmul(
                    l1t[:, 12 + 500 * c:12 + 500 * (c + 1)],
                    lhsT=w1a_sb[64 * c:64 * (c + 1),
                                128 * h:128 * (h + 1)],
                    rhs=(xrow[b][64 * c:64 * (c + 1), 0:KC] if g == 0 else
                         xts[g][64 * c:64 * (c + 1), j, 0:KC]),
                    start=True, stop=True)
            bias = hs_sb[:, 32 * h + b:32 * h + b + 1]
            if b < 3:
                # cold-clock phase: the merged ep is a ~3us ring through a
                # cold ScalarE; two 500-col eps on OPPOSITE engines run in
                # parallel and halve the ring until the governor warms up.
                for c in range(2):
                    ep(h1[:, 12 + 500 * c:12 + 500 * (c + 1)],
                       l1t[:, 12 + 500 * c:12 + 500 * (c + 1)],
                       bias, relu=True, eng=(h + c) % 2)
            else:
                ep(h1[:, 12:EPW], l1t[:, 12:EPW], bias, relu=True, eng=h)
            return h1

        def stage_l2(b, h1t):
            # merged [128,1024] PSUM tile; hh-outer so each W2 half is one
            # LDWEIGHTS; ONE merged relu+bias epilogue.
            h2 = h2p.tile([128, 1024], bf16, tag="h2")
            for c in range(2):
                l2t = l2p.tile([128, 512], f32, tag="l2")
                for hh in range(2):
                    nc.tensor.matmul(
                        l2t[:, 0:KC],
                        lhsT=w2_sb[:, hh, :],
                        rhs=h1t[hh][:, 12 + 500 * c:12 + 500 * (c + 1)],
                        start=(hh == 0), stop=(hh == 1))
                # per-chunk single-bank tile + epilogue right after its
                # 2-matmul group closes: independent tiles keep the ring
                # short (tile-granularity W-after-R tracking serializes
                # sub-slices of one tile).
                ep(h2[:, KP * c:KP * c + KC], l2t[:, 0:KC],
                   b2_sb[:, 0:1], relu=True, eng=c)
            return h2

        # L3: score row (b, ch) lands on PSUM partition p = 2b+ch via a
        # full-array matmul whose stationary is zeros except col p = w3
        # (sliding window of w3s_sb); all 64 matmuls accumulate into one
        # bank (+0.0 on every other partition), one copy + one DMA out.
        # Two sequential 16-step accumulation groups reuse the single L3
        # bank (pool W-after-R covers the handoff), so the first half's
        # copy + 64KB DMA overlap steady state at step ~17 instead of
        # sitting on the tail.
        l3state = {}

        def stage_l3(b, h2):
            half, bh = divmod(b, BL // 2)
            if bh == 0:
                l3state[0] = l3p.tile([128, 512], f32, tag="l3",
                                      name=f"l3bank{half}")
            l3b = l3state[0]
            for ch in range(2):
                p = 2 * bh + ch
                nc.tensor.matmul(
                    l3b[:, 0:KC],
                    lhsT=w3s_sb[:, 63 - p:191 - p],
                    rhs=h2[:, 0:KC] if ch == 0 else h2[:, KP:KP + KC],
                    start=(p == 0), stop=(p == BL - 1))
            if bh == BL // 2 - 1:
                if half == 0:
                    l3state["osb"] = osp.tile([2 * BL, 500], f32, tag="osb",
                                              name="osb")
                osb = l3state["osb"]
                rows = slice(BL * half, BL * (half + 1))
                with tc.high_priority():
                    if half == 0:
                        # mid-run: ScE identity copy (its slack absorbs it)
                        nc.scalar.activation(osb[rows, :], l3b[0:BL, 0:KC],
                                             AF.Identity)
                    else:
                        nc.vector.tensor_scalar(osb[rows, :],
                                                l3b[0:BL, 0:KC], 0.0,
                                                None, ALU.add)
                nc.sync.dma_start(
                    out[16 * half:16 * (half + 1)]
                    .rearrange("b (c y) -> (b c) y", c=2), osb[rows, :])

        pend = {}
        pend2 = {}
        for s in range(BL + 2):
            if s < BL:
                if s % GRP == 0:
                    stage_xt(s // GRP + 1)
                h1a = stage_l1h(s, 0)
                h1b = stage_l1h(s, 1)
            if s - 1 in pend:
                pend2[s - 1] = stage_l2(s - 1, pend.pop(s - 1))
            if s - 2 in pend2:
                stage_l3(s - 2, pend2.pop(s - 2))
            if s < BL:
                pend[s] = [h1a, h1b]

    nc.compile()
    return nc


def _prep_inputs(state_embed, action_feats, W1, b1, W2, b2, W3, b3):
    import ml_dtypes
    bf = ml_dtypes.bfloat16
    f4 = lambda x: np.ascontiguousarray(np.asarray(x, dtype=np.float32))
    state_embed, action_feats = f4(state_embed), f4(action_feats)
    W1, b1, W2, b2, W3, b3 = map(f4, (W1, b1, W2, b2, W3, b3))

    W1s, W1a = W1[:SD], W1[SD:]
    w1s_h = np.concatenate([W1s[c * 128:(c + 1) * 128] for c in range(4)],
                           axis=1).astype(bf)                # [128, 1024]
    w1a_h = np.concatenate([W1a, W1a], axis=0).astype(bf)    # [128, 256]
    w2_h = np.concatenate([W2[:128], W2[128:]], axis=1).astype(bf)
    w3s_h = np.zeros((G, 192), dtype=np.float32)
    w3s_h[:, 63] = W3[:, 0]
    w3s_h = w3s_h.astype(bf)
    biases = np.ascontiguousarray(np.concatenate(
        [b1.reshape(2, 128).T, b2.reshape(G, 1)], axis=1))   # [128, 3] f32

    in_maps = []
    for ci in range(NCORES):
        sl = slice(ci * BL, (ci + 1) * BL)
        aft = action_feats[sl].transpose(0, 2, 1)            # [BL, 64, 1000]
        a2_h = np.zeros((BL, 128, KP), dtype=bf)
        a2_h[:, 0:64, 0:KC] = aft[:, :, :KC].astype(bf)
        a2_h[:, 64:128, 0:KC] = aft[:, :, KC:].astype(bf)
        a2_h = np.ascontiguousarray(
            a2_h.reshape(BL // GRP, GRP, 128, KP).transpose(0, 2, 1, 3)
            .reshape(BL // GRP, 128, GRP * KP))
        st = state_embed[sl].T.astype(bf)                    # [512, BL]
        s2_h = np.concatenate([st[c * 128:(c + 1) * 128] for c in range(4)],
                              axis=1)                        # [128, 128]
        consts = np.ascontiguousarray(np.concatenate(
            [w1a_h, w2_h, w3s_h, s2_h, w1s_h], axis=1))
        assert consts.shape == (128, NCONST), consts.shape
        in_maps.append({"a2": a2_h, "consts": consts, "biases": biases})
    return in_maps, float(b3.reshape(-1)[0])


def kernel(state_embed, action_feats, W1, b1, W2, b2, W3, b3):
    global LAST_EXEC_NS
    from concourse.bass_utils import run_bass_kernel_spmd

    if "nc" not in _CACHE:
        _CACHE["nc"] = _build_nc()
    nc = _CACHE["nc"]

    in_maps, b3v = _prep_inputs(state_embed, action_feats, W1, b1, W2, b2,
                                W3, b3)
    trace = bool(int(os.environ.get("ACTOR_KERNEL_TRACE", "0")))
    res = run_bass_kernel_spmd(nc, in_maps, core_ids=list(range(NCORES)),
                               trace=trace)
    LAST_EXEC_NS = res.exec_time_ns
    outs = [np.asarray(res.results[i]["out"]) for i in range(NCORES)]
    return np.ascontiguousarray(
        (np.concatenate(outs, axis=0) + b3v).astype(np.float32))
